# revision 24
# baseline (speedup 1.0000x reference)
"""Trainium2 Bass kernel for CAGKE (Gaussian-kernel spike embedding).

Math: psedu[t] = sum_d softmax(weight)[d] * (spikes (*) K_d)[t] + noise[t],
followed by global min-max normalization. Because the softmax weights do not
depend on t, the weighted sum over the D=128 kernel bank commutes with the
convolution: psedu = spikes (*) kbar + noise, where
kbar(delta) = sum_d sw_d * (C/sigma_d) * exp(-(delta-1)^2 / (2 sigma_d^2)).

The Gaussian bank (sigma <= 3.0) underflows f32 beyond |delta-1| ~ 44, so a
511-tap kbar (delta in [-255, 255]) reproduces the reference conv bit-tight.
The conv is computed as 3 banded 128x128 matmuls on the tensor engine against
a Toeplitz matrix built from kbar via a DRAM bounce (overlapping-window reads
are only well-defined on the DRAM side of a DMA, and the BIR verifier only
allows ascending outer strides, hence the partition-flipped contraction with
the spikes flipped to match by an exchange-matrix matmul).

Two scaling tricks keep the critical path short: min-max normalization is
invariant under positive scaling, so the softmax denominator 1/esum is never
applied to the kernel bank -- the noise is scaled by esum instead
(psedu_scaled = esum * psedu_true), off the critical path.

All 8 cores run the identical replicated program (total I/O is ~100KB, far
below the point where sharding would beat collective/sync overhead); the
host takes core 0's output.
"""

import os
import sys

for _p in ("/opt/trn_rl_repo", "/root/.axon_site/_ro/trn_rl_repo"):
    if os.path.isdir(_p) and _p not in sys.path:
        sys.path.insert(0, _p)

import numpy as np

T = 8192  # in_length
D = 128  # embed_dim (kernel bank size)
GAUSS_C = 0.39894228  # 1/sqrt(2*pi) as hardcoded in the source module
NCORES = 8
COLS = T // 128  # 64 columns of 128 contiguous time steps
KW = 511  # kbar taps, delta in [-255, 255]
J0, JW = 191, 130  # nonzero kbar window: j in [191, 321) -> delta in [-64, 65]

_CACHE = {}


def _build_bass():
    import concourse.bass as bass
    import concourse.tile as tile
    from concourse import bacc, mybir

    f32 = mybir.dt.float32
    nc = bacc.Bacc("TRN2", target_bir_lowering=False, debug=False, num_devices=NCORES)

    x_d = nc.dram_tensor("X", [1, T], f32, kind="ExternalInput")
    w_d = nc.dram_tensor("weight", [1, D], f32, kind="ExternalInput")
    n_d = nc.dram_tensor("noise", [1, T], f32, kind="ExternalInput")
    s_d = nc.dram_tensor("sigma", [D], f32, kind="ExternalInput")
    o_d = nc.dram_tensor("out", [1, T], f32, kind="ExternalOutput")

    kb_d = nc.dram_tensor("kb_scratch", [KW], f32)  # internal DRAM bounce

    debug = bool(os.environ.get("KERNEL_DEBUG_TAPS"))
    dbg = {}
    if debug:
        dbg["L"] = nc.dram_tensor("dbg_L", [128, 384], f32, kind="ExternalOutput")
        dbg["kb"] = nc.dram_tensor("dbg_kb", [1, KW], f32, kind="ExternalOutput")
        dbg["spf"] = nc.dram_tensor("dbg_spf", [128, COLS + 2], f32, kind="ExternalOutput")
        dbg["conv"] = nc.dram_tensor("dbg_conv", [128, COLS], f32, kind="ExternalOutput")
        dbg["psrm"] = nc.dram_tensor("dbg_psrm", [COLS, 128], f32, kind="ExternalOutput")
        dbg["coeff"] = nc.dram_tensor("dbg_coeff", [D, 1], f32, kind="ExternalOutput")

    with tile.TileContext(nc) as tc:
        with (
            tc.tile_pool(name="sb", bufs=1) as sb,
            tc.tile_pool(name="ps", bufs=1, space="PSUM") as ps,
        ):
            # ---- input DMAs, most-critical first; two HWDGE rings ----
            # sync(SP) ring: sigma, weight, X feed the serial kbar chain
            sig = sb.tile([D, 1], f32)
            nc.sync.dma_start(out=sig[:], in_=s_d.ap().unsqueeze(1))
            w_row = sb.tile([1, D], f32)
            nc.sync.dma_start(out=w_row[:], in_=w_d.ap())
            m_x = sb.tile([COLS, 128], f32)
            nc.sync.dma_start(
                out=m_x[:], in_=x_d.ap().rearrange("a (c p) -> (a c) p", p=128)
            )
            # scalar(ACT) ring: noise + kbar edge zeros are off-critical
            nrm = sb.tile([COLS, 128], f32)
            nc.scalar.dma_start(
                out=nrm[:], in_=n_d.ap().rearrange("a (c p) -> (a c) p", p=128)
            )
            zer = sb.tile([1, KW - JW], f32)
            nc.vector.memset(zer[:], 0.0)
            nc.scalar.dma_start(out=kb_d.ap()[0:J0].unsqueeze(0), in_=zer[:, 0:J0])
            nc.scalar.dma_start(
                out=kb_d.ap()[J0 + JW : KW].unsqueeze(0),
                in_=zer[:, 0 : KW - J0 - JW],
            )

            # ---- constants (no input deps; scheduler runs them early) ----
            one1 = sb.tile([1, 1], f32)
            nc.vector.memset(one1[:], 1.0)
            id64 = sb.tile([COLS, COLS], f32)
            nc.gpsimd.memset(id64[:], 0.0)
            nc.gpsimd.affine_select(
                out=id64[:], in_=id64[:], compare_op=mybir.AluOpType.not_equal,
                fill=1.0, base=0, pattern=[[-1, COLS]], channel_multiplier=1,
            )
            id128 = sb.tile([128, 128], f32)
            nc.gpsimd.memset(id128[:], 0.0)
            nc.gpsimd.affine_select(
                out=id128[:], in_=id128[:], compare_op=mybir.AluOpType.not_equal,
                fill=1.0, base=0, pattern=[[-1, 128]], channel_multiplier=1,
            )
            jx128 = sb.tile([128, 128], f32)  # exchange matrix (anti-diagonal)
            nc.gpsimd.memset(jx128[:], 0.0)
            nc.gpsimd.affine_select(
                out=jx128[:], in_=jx128[:], compare_op=mybir.AluOpType.not_equal,
                fill=1.0, base=-127, pattern=[[1, 128]], channel_multiplier=1,
            )
            ones2 = sb.tile([2, COLS], f32)
            nc.vector.memset(ones2[:], 1.0)
            # mconst = [[1, 0], [1, -1]]: one matmul then maps
            # g = [gmax, -gmin] to [range, gmin] broadcast over partitions
            mconst = sb.tile([2, 2], f32)
            nc.gpsimd.memset(mconst[:], 1.0)
            nc.gpsimd.affine_select(
                out=mconst[:], in_=mconst[:], compare_op=mybir.AluOpType.not_equal,
                fill=0.0, base=-1, pattern=[[1, 2]], channel_multiplier=2,
            )  # zero at (r0, f1)
            nc.gpsimd.affine_select(
                out=mconst[:], in_=mconst[:], compare_op=mybir.AluOpType.not_equal,
                fill=-1.0, base=-3, pattern=[[1, 2]], channel_multiplier=2,
            )  # -1 at (r1, f1)
            jj = sb.tile([D, JW], f32)  # j - 256, exact in f32
            nc.gpsimd.iota(
                jj[:], pattern=[[1, JW]], base=J0 - 256, channel_multiplier=0,
                allow_small_or_imprecise_dtypes=True,
            )
            dsq = sb.tile([D, JW], f32)
            nc.scalar.activation(
                out=dsq[:], in_=jj[:], func=mybir.ActivationFunctionType.Square,
                bias=0.0, scale=1.0,
            )  # (j - 256)^2

            # ---- sigma-derived per-partition scalars (d on partitions) ----
            s2 = sb.tile([D, 1], f32)
            nc.vector.tensor_mul(s2[:], sig[:], sig[:])
            inv_s2 = sb.tile([D, 1], f32)
            nc.vector.reciprocal(inv_s2[:], s2[:])
            nhalf = sb.tile([D, 1], f32)  # -1/(2 sigma^2)
            nc.scalar.mul(nhalf[:], inv_s2[:], -0.5)
            inv_sig = sb.tile([D, 1], f32)
            nc.vector.reciprocal(inv_sig[:], sig[:])
            inv_sig_c = sb.tile([D, 1], f32)  # C / sigma
            nc.scalar.mul(inv_sig_c[:], inv_sig[:], GAUSS_C)
            expm = sb.tile([D, JW], f32)
            nc.scalar.activation(
                out=expm[:], in_=dsq[:], func=mybir.ActivationFunctionType.Exp,
                bias=0.0, scale=nhalf[:, 0:1],
            )  # per-sigma gaussian row

            # ---- softmax numerator; denominator folds into the noise ----
            wmax = sb.tile([1, 1], f32)
            nc.vector.tensor_reduce(
                out=wmax[:], in_=w_row[:], axis=mybir.AxisListType.X,
                op=mybir.AluOpType.max,
            )
            # the ACT instruction encoding only has one sync-wait slot, so
            # keep its cross-engine deps to a single engine (DVE)
            w_shift = sb.tile([1, D], f32)
            nc.vector.tensor_scalar(
                out=w_shift[:], in0=w_row[:], scalar1=wmax[:, 0:1], scalar2=None,
                op0=mybir.AluOpType.subtract,
            )
            e_row = sb.tile([1, D], f32)
            esum = sb.tile([1, 1], f32)
            nc.scalar.activation(
                out=e_row[:], in_=w_shift[:], func=mybir.ActivationFunctionType.Exp,
                bias=0.0, scale=1.0, accum_out=esum[:, 0:1],
            )  # exp(w - max), esum = sum
            e_ps = ps.tile([D, 1], f32, tag="ps_a")  # exp weights onto partitions
            nc.tensor.matmul(e_ps[:], lhsT=e_row[:], rhs=one1[:], start=True, stop=True)
            coeff = sb.tile([D, 1], f32)  # e_d * C / sigma_d  (unnormalized)
            nc.vector.tensor_mul(coeff[:], e_ps[:], inv_sig_c[:])

            # ---- kbar window and Toeplitz bank via DRAM bounce ----
            kb_ps = ps.tile([1, JW], f32, tag="ps_b")
            nc.tensor.matmul(kb_ps[:], lhsT=coeff[:], rhs=expm[:], start=True, stop=True)
            kb_sb = sb.tile([1, JW], f32)
            nc.vector.tensor_copy(kb_sb[:], kb_ps[:])
            nc.sync.dma_start(
                out=kb_d.ap()[J0 : J0 + JW].unsqueeze(0), in_=kb_sb[:],
                single_packet=True,
            )
            # L[qt, p'] = kbar[qt + p'] (flipped contraction index qt = 127-q;
            # the spike operand is partition-flipped to match). All-positive
            # strides; each partition reads a contiguous 1536B row.
            L = sb.tile([128, 384], f32)
            ksrc = bass.AP(
                tensor=kb_d.ap().tensor, offset=0, ap=[[1, 128], [1, 384]]
            )
            nc.sync.dma_start(out=L[:], in_=ksrc)

            # ---- spikes: threshold, PE transpose, partition flip ----
            spk = sb.tile([COLS, 128], f32)
            nc.vector.tensor_scalar(
                out=spk[:], in0=m_x[:], scalar1=0.5, scalar2=None,
                op0=mybir.AluOpType.is_gt,
            )
            sp_ps = ps.tile([128, COLS], f32, tag="ps_c")
            nc.tensor.transpose(sp_ps[:], spk[:], id64[:])
            sp_sb = sb.tile([128, COLS], f32)
            nc.vector.tensor_copy(sp_sb[:], sp_ps[:])
            spf_ps = ps.tile([128, COLS], f32, tag="ps_d")
            nc.tensor.matmul(
                spf_ps[:], lhsT=jx128[:], rhs=sp_sb[:], start=True, stop=True
            )  # partition-flip: spf_ps[qt, c] = spikes[128c + 127 - qt]
            spf = sb.tile([128, COLS + 2], f32)  # zero halo columns at 0 and 65
            nc.gpsimd.memset(spf[:, 0:1], 0.0)
            nc.gpsimd.memset(spf[:, COLS + 1 : COLS + 2], 0.0)
            nc.vector.tensor_copy(spf[:, 1 : COLS + 1], spf_ps[:])

            # ---- banded conv: out[:, c] = sum_b A_b @ Sp[:, c+b] ----
            conv_ps = ps.tile([128, COLS], f32)
            for k, b in ((0, 1), (1, 0), (2, -1)):
                nc.tensor.matmul(
                    conv_ps[:],
                    lhsT=L[:, 128 * k : 128 * (k + 1)],
                    rhs=spf[:, 1 + b : COLS + 1 + b],
                    start=(k == 0),
                    stop=(k == 2),
                )
            conv_sb = sb.tile([128, COLS], f32)
            nc.vector.tensor_copy(conv_sb[:], conv_ps[:])

            # ---- back to row-major [64, 128], add esum-scaled noise ----
            ct_ps = ps.tile([COLS, 128], f32)
            nc.tensor.transpose(ct_ps[:], conv_sb[:], id128[:])
            es_ps = ps.tile([COLS, 1], f32, tag="ps_d")  # esum bcast, 64 parts
            nc.tensor.matmul(
                es_ps[:], lhsT=ones2[0:1, :], rhs=esum[:], start=True, stop=True
            )
            es_col = sb.tile([COLS, 1], f32)
            nc.vector.tensor_copy(es_col[:], es_ps[:])
            ps_rm = sb.tile([COLS, 128], f32)
            nc.vector.scalar_tensor_tensor(
                out=ps_rm[:], in0=nrm[:], scalar=es_col[:, 0:1], in1=ct_ps[:],
                op0=mybir.AluOpType.mult, op1=mybir.AluOpType.add,
            )  # esum * (conv + noise_true) up to the global scale

            # ---- global min/max + normalize ----
            # Per-partition stats packed as [max, -min], PE-transposed to one
            # row pair, reduced along free; then gg = mconst * g and one
            # matmul against ones broadcasts [range, gmin] to all partitions.
            pk = sb.tile([COLS, 2], f32)
            nc.vector.tensor_reduce(
                out=pk[:, 0:1], in_=ps_rm[:], axis=mybir.AxisListType.X,
                op=mybir.AluOpType.max,
            )
            nc.vector.tensor_reduce(
                out=pk[:, 1:2], in_=ps_rm[:], axis=mybir.AxisListType.X,
                op=mybir.AluOpType.min, negate=True,
            )
            pk_ps = ps.tile([2, COLS], f32, tag="ps_c")
            nc.tensor.transpose(pk_ps[:], pk[:], id64[:])
            g = sb.tile([2, 1], f32)
            nc.vector.tensor_reduce(
                out=g[:], in_=pk_ps[:], axis=mybir.AxisListType.X,
                op=mybir.AluOpType.max,
            )  # g[0] = gmax, g[1] = -gmin
            gg = sb.tile([2, 2], f32)
            nc.vector.tensor_scalar_mul(gg[:], in0=mconst[:], scalar1=g[:, 0:1])
            stat_ps = ps.tile([COLS, 2], f32, tag="ps_a")
            nc.tensor.matmul(stat_ps[:], lhsT=ones2[:], rhs=gg[:], start=True, stop=True)
            inv_rng = sb.tile([COLS, 1], f32)
            nc.vector.reciprocal(inv_rng[:], stat_ps[:, 0:1])
            outt = sb.tile([COLS, 128], f32)
            nc.vector.tensor_scalar(
                out=outt[:], in0=ps_rm[:], scalar1=stat_ps[:, 1:2],
                scalar2=inv_rng[:, 0:1], op0=mybir.AluOpType.subtract,
                op1=mybir.AluOpType.mult,
            )
            nc.sync.dma_start(
                out=o_d.ap().rearrange("a (c p) -> (a c) p", p=128), in_=outt[:]
            )

            if debug:
                nc.sync.dma_start(out=dbg["L"].ap(), in_=L[:])
                nc.sync.dma_start(out=dbg["kb"].ap(), in_=kb_d.ap().unsqueeze(0))
                nc.sync.dma_start(out=dbg["spf"].ap(), in_=spf[:])
                nc.sync.dma_start(out=dbg["conv"].ap(), in_=conv_sb[:])
                nc.sync.dma_start(out=dbg["psrm"].ap(), in_=ps_rm[:])
                nc.sync.dma_start(out=dbg["coeff"].ap(), in_=coeff[:])

    nc.compile()
    return nc


def _get_nc():
    if "nc" not in _CACHE:
        _CACHE["nc"] = _build_bass()
    return _CACHE["nc"]


def _run(in_map, trace=False, **kwargs):
    from concourse.bass_utils import run_bass_kernel_spmd

    nc = _get_nc()
    return run_bass_kernel_spmd(
        nc, [in_map] * NCORES, core_ids=list(range(NCORES)), trace=trace, **kwargs
    )


def kernel(X, weight, noise, sigma):
    in_map = {
        "X": np.ascontiguousarray(X, dtype=np.float32).reshape(1, T),
        "weight": np.ascontiguousarray(weight, dtype=np.float32).reshape(1, D),
        "noise": np.ascontiguousarray(noise, dtype=np.float32).reshape(1, T),
        "sigma": np.ascontiguousarray(sigma, dtype=np.float32).reshape(D),
    }
    res = _run(in_map).results
    return res[0]["out"].reshape(1, T)
